# revision 19
# baseline (speedup 1.0000x reference)
"""Trainium2 Bass kernel for MF embedding-lookup + dot-product scoring.

out[u, i] = dot(user_hiddens[user_ids[u]], item_hiddens[item_ids[i]])

Sharding: 2D over 8 cores - 4 user groups (1024 users) x 2 item groups
(2048 items); tables replicated in every core's HBM.

The hard constraint on TRN2 is SWDGE descriptor generation: every
indirect-DMA call costs ~1us fixed + ~2.5ns/row on the single GpSimd
queue, one 128-row call per user/item tile (the HW ucode supports one
index per partition per call; the custom dma_gather ucode is ~10ns/idx -
slower). 24 calls/core is the minimum for a rectangular sharding, so the
kernel hides everything else behind that ~28us serialized gather stream:
  - user and item gathers interleaved (u,i,i x 8) so transposes, matmuls
    and output DMA start after the third call
  - PE transpose to [64, batch]; psum staged 2 item tiles per bank
  - plain bf16 matmuls (tolerance 2e-2; bf16 on positive uniform data
    gives ~4e-3), N=512 f32 PSUM blocks, copies cast to bf16 alternating
    scalar/vector
  - output written as bf16 [1024, 2048] per core, flushed in halves per
    user tile; host casts back to f32
  - warm-up matmuls keep the PE out of its low p-state
"""

import numpy as np

import concourse.bacc as bacc
import concourse.bass as bass
import concourse.mybir as mybir
import concourse.tile as tile
from concourse.bass_utils import run_bass_kernel_spmd
from concourse.masks import make_identity

NUM_USERS = 1_000_000
NUM_ITEMS = 100_000
D = 64
BU = 4096
BI = 4096
N_CORES = 8
RU = 4              # user groups
RI = 2              # item groups
UC = BU // RU       # users per core = 1024
IC = BI // RI       # items per core = 2048
P = 128
UT = UC // P        # user tiles per core = 8
IT = IC // P        # item tiles per core = 16
NBLK = 512          # matmul moving free dim
NB = IC // NBLK     # item blocks = 4

_cache = {}


def _build():
    nc = bacc.Bacc()
    ut_dram = nc.dram_tensor(
        "user_table", [NUM_USERS, D], mybir.dt.float32, kind="ExternalInput"
    )
    it_dram = nc.dram_tensor(
        "item_table", [NUM_ITEMS, D], mybir.dt.float32, kind="ExternalInput"
    )
    # ids[p, 0:8] = user tile ids, ids[p, 8:24] = item tile ids
    ids_dram = nc.dram_tensor(
        "ids", [P, UT + IT], mybir.dt.int32, kind="ExternalInput"
    )
    out_dram = nc.dram_tensor(
        "out", [UC, IC], mybir.dt.bfloat16, kind="ExternalOutput"
    )

    f32 = mybir.dt.float32
    bf16 = mybir.dt.bfloat16

    with tile.TileContext(nc) as tc:
        with (
            tc.tile_pool(name="const", bufs=1) as constp,
            tc.tile_pool(name="idx", bufs=1) as idxp,
            tc.tile_pool(name="gath", bufs=1) as gathp,
            tc.tile_pool(name="stk", bufs=1) as stkp,
            tc.tile_pool(name="tp", bufs=2, space="PSUM") as tpp,
            tc.tile_pool(name="mm", bufs=2, space="PSUM") as mmp,
            tc.tile_pool(name="outp", bufs=1) as outp,
        ):
            ident = constp.tile([P, P], f32)
            make_identity(nc, ident[:])

            ids = idxp.tile([P, UT + IT], mybir.dt.int32)
            nc.sync.dma_start(out=ids[:], in_=ids_dram[:])

            # PE warm-up to hold p-state through the gather phase
            for w in range(6):
                junk = mmp.tile([P, NBLK], f32)
                nc.tensor.matmul(
                    junk[:, 0:P], lhsT=ident[:], rhs=ident[:],
                    start=True, stop=True, skip_group_check=True,
                )

            gu = [gathp.tile([P, D], f32, name=f"gu{k}") for k in range(UT)]
            gv = [gathp.tile([P, D], f32, name=f"gv{k}") for k in range(IT)]

            def gather(dst, table, col):
                nc.gpsimd.indirect_dma_start(
                    out=dst[:],
                    out_offset=None,
                    in_=table[:],
                    in_offset=bass.IndirectOffsetOnAxis(
                        ap=ids[:, col : col + 1], axis=0
                    ),
                )

            ustack = stkp.tile([D, UC], bf16)   # [64, 1024]
            vstack = stkp.tile([D, IC], bf16)   # [64, 2048]

            cp_rot = [0]

            def copy(dst, src):
                e = cp_rot[0] % 2
                cp_rot[0] += 1
                if e == 0:
                    nc.scalar.copy(out=dst, in_=src)
                else:
                    nc.vector.tensor_copy(out=dst, in_=src)

            def u_transpose(t):
                ps = tpp.tile([D, NBLK], f32)
                nc.tensor.transpose(ps[:, 0:P], gu[t][:], ident[:])
                copy(ustack[:, t * P : (t + 1) * P], ps[:, 0:P])

            def i_transpose_pair(j):
                # item tiles 2j, 2j+1 -> one psum tile, one copy
                ps = tpp.tile([D, NBLK], f32)
                for q in range(2):
                    nc.tensor.transpose(
                        ps[:, q * P : (q + 1) * P], gv[2 * j + q][:], ident[:]
                    )
                copy(vstack[:, 2 * j * P : (2 * j + 2) * P], ps[:, 0 : 2 * P])

            ot = [outp.tile([P, IC], bf16, name=f"ot{k}") for k in range(UT)]

            def mm(t, n):
                po = mmp.tile([P, NBLK], f32)
                nc.tensor.matmul(
                    po[:],
                    lhsT=ustack[:, t * P : (t + 1) * P],
                    rhs=vstack[:, n * NBLK : (n + 1) * NBLK],
                    start=True,
                    stop=True,
                )
                copy(ot[t][:, n * NBLK : (n + 1) * NBLK], po[:])
                # flush [0:1024) after block 1, then each later block alone,
                # spreading output DMA instead of bunching it in the tail
                if n >= 1:
                    sl = slice(0 if n == 1 else n * NBLK, (n + 1) * NBLK)
                    nc.sync.dma_start(
                        out=out_dram[t * P : (t + 1) * P, sl],
                        in_=ot[t][:, sl],
                    )

            # --- gather schedule: item tiles 0-7 first (unlocks matmul
            # blocks 0-1 and the first output flushes early), then users
            # trickled between the remaining item tiles, ending on users so
            # the post-gather tail is small.
            SEQ = (
                [("i", j) for j in range(8)]
                + [("u", 0)]
                + [("i", 8), ("i", 9), ("u", 1), ("i", 10), ("i", 11),
                   ("u", 2), ("i", 12), ("i", 13), ("u", 3), ("i", 14),
                   ("i", 15), ("u", 4), ("u", 5), ("u", 6), ("u", 7)]
            )
            for kind, j in SEQ:
                if kind == "u":
                    gather(gu[j], ut_dram, j)
                else:
                    gather(gv[j], it_dram, UT + j)

            done = set()

            def emit_ready(users_done, items_done):
                blocks = min(NB, items_done // 4)
                for t in range(users_done):
                    for n in range(blocks):
                        if (t, n) not in done:
                            done.add((t, n))
                            mm(t, n)

            users_done = items_done = 0
            for kind, j in SEQ:
                if kind == "u":
                    u_transpose(j)
                    users_done = j + 1
                else:
                    if j % 2 == 1:
                        i_transpose_pair(j // 2)
                    items_done = j + 1
                emit_ready(users_done, items_done)

    nc.finalize()
    return nc


def kernel(user_hiddens, item_hiddens, user_ids, item_ids, **_):
    user_hiddens = np.ascontiguousarray(user_hiddens, dtype=np.float32)
    item_hiddens = np.ascontiguousarray(item_hiddens, dtype=np.float32)
    user_ids = np.asarray(user_ids)
    item_ids = np.asarray(item_ids)

    if "nc" not in _cache:
        _cache["nc"] = _build()
    nc = _cache["nc"]

    in_maps = []
    for c in range(N_CORES):
        cu, ci = divmod(c, RI)
        uc = user_ids[cu * UC : (cu + 1) * UC]
        icd = item_ids[ci * IC : (ci + 1) * IC]
        ids_t = np.empty((P, UT + IT), dtype=np.int32)
        ids_t[:, :UT] = uc.astype(np.int32).reshape(UT, P).T
        ids_t[:, UT:] = icd.astype(np.int32).reshape(IT, P).T
        in_maps.append(
            {
                "user_table": user_hiddens,
                "item_table": item_hiddens,
                "ids": np.ascontiguousarray(ids_t),
            }
        )

    res = run_bass_kernel_spmd(nc, in_maps, list(range(N_CORES)))
    out = np.empty((BU, BI), dtype=np.float32)
    for c in range(N_CORES):
        cu, ci = divmod(c, RI)
        out[cu * UC : (cu + 1) * UC, ci * IC : (ci + 1) * IC] = np.asarray(
            res.results[c]["out"]
        ).astype(np.float32)
    return out


# revision 20
# speedup vs baseline: 1.0647x; 1.0647x over previous
"""Trainium2 Bass kernel for MF embedding-lookup + dot-product scoring.

out[u, i] = dot(user_hiddens[user_ids[u]], item_hiddens[item_ids[i]])

Sharding: 2D over 8 cores - 4 user groups (1024 users) x 2 item groups
(2048 items); tables replicated in every core's HBM.

The hard constraint on TRN2 is SWDGE descriptor generation: every
indirect-DMA call costs ~1us fixed + ~2.5ns/row on the single GpSimd
queue, one 128-row call per user/item tile (the HW ucode supports one
index per partition per call; the custom dma_gather ucode is ~10ns/idx -
slower). 24 calls/core is the minimum for a rectangular sharding, so the
kernel hides everything else behind that ~28us serialized gather stream:
  - user and item gathers interleaved (u,i,i x 8) so transposes, matmuls
    and output DMA start after the third call
  - PE transpose to [64, batch]; psum staged 2 item tiles per bank
  - plain bf16 matmuls (tolerance 2e-2; bf16 on positive uniform data
    gives ~4e-3), N=512 f32 PSUM blocks, copies cast to bf16 alternating
    scalar/vector
  - output written as bf16 [1024, 2048] per core, flushed in halves per
    user tile; host casts back to f32
  - warm-up matmuls keep the PE out of its low p-state
"""

import numpy as np

import concourse.bacc as bacc
import concourse.bass as bass
import concourse.mybir as mybir
import concourse.tile as tile
from concourse.bass_utils import run_bass_kernel_spmd
from concourse.masks import make_identity

NUM_USERS = 1_000_000
NUM_ITEMS = 100_000
D = 64
BU = 4096
BI = 4096
N_CORES = 8
RU = 4              # user groups
RI = 2              # item groups
UC = BU // RU       # users per core = 1024
IC = BI // RI       # items per core = 2048
P = 128
UT = UC // P        # user tiles per core = 8
IT = IC // P        # item tiles per core = 16
NBLK = 512          # matmul moving free dim
NB = IC // NBLK     # item blocks = 4

_cache = {}


def _build():
    nc = bacc.Bacc()
    ut_dram = nc.dram_tensor(
        "user_table", [NUM_USERS, D], mybir.dt.float32, kind="ExternalInput"
    )
    it_dram = nc.dram_tensor(
        "item_table", [NUM_ITEMS, D], mybir.dt.float32, kind="ExternalInput"
    )
    # ids[p, 0:8] = user tile ids, ids[p, 8:24] = item tile ids
    ids_dram = nc.dram_tensor(
        "ids", [P, UT + IT], mybir.dt.int32, kind="ExternalInput"
    )
    out_dram = nc.dram_tensor(
        "out", [UC, IC], mybir.dt.bfloat16, kind="ExternalOutput"
    )

    f32 = mybir.dt.float32
    bf16 = mybir.dt.bfloat16

    with tile.TileContext(nc) as tc:
        with (
            tc.tile_pool(name="const", bufs=1) as constp,
            tc.tile_pool(name="idx", bufs=1) as idxp,
            tc.tile_pool(name="gath", bufs=1) as gathp,
            tc.tile_pool(name="stk", bufs=1) as stkp,
            tc.tile_pool(name="tp", bufs=2, space="PSUM") as tpp,
            tc.tile_pool(name="mm", bufs=2, space="PSUM") as mmp,
            tc.tile_pool(name="outp", bufs=1) as outp,
        ):
            ident = constp.tile([P, P], f32)
            make_identity(nc, ident[:])

            ids = idxp.tile([P, UT + IT], mybir.dt.int32)
            nc.sync.dma_start(out=ids[:], in_=ids_dram[:])

            # PE warm-up to hold p-state through the gather phase
            for w in range(6):
                junk = mmp.tile([P, NBLK], f32)
                nc.tensor.matmul(
                    junk[:, 0:P], lhsT=ident[:], rhs=ident[:],
                    start=True, stop=True, skip_group_check=True,
                )

            gu = [gathp.tile([P, D], f32, name=f"gu{k}") for k in range(UT)]
            gv = [gathp.tile([P, D], f32, name=f"gv{k}") for k in range(IT)]

            def gather(dst, table, col):
                nc.gpsimd.indirect_dma_start(
                    out=dst[:],
                    out_offset=None,
                    in_=table[:],
                    in_offset=bass.IndirectOffsetOnAxis(
                        ap=ids[:, col : col + 1], axis=0
                    ),
                )

            ustack = stkp.tile([D, UC], bf16)   # [64, 1024]
            vstack = stkp.tile([D, IC], bf16)   # [64, 2048]

            cp_rot = [0]

            def copy(dst, src):
                e = cp_rot[0] % 2
                cp_rot[0] += 1
                if e == 0:
                    nc.scalar.copy(out=dst, in_=src)
                else:
                    nc.vector.tensor_copy(out=dst, in_=src)

            def u_transpose(t):
                ps = tpp.tile([D, NBLK], f32)
                nc.tensor.transpose(ps[:, 0:P], gu[t][:], ident[:])
                copy(ustack[:, t * P : (t + 1) * P], ps[:, 0:P])

            def i_transpose_pair(j):
                # item tiles 2j, 2j+1 -> one psum tile, one copy
                ps = tpp.tile([D, NBLK], f32)
                for q in range(2):
                    nc.tensor.transpose(
                        ps[:, q * P : (q + 1) * P], gv[2 * j + q][:], ident[:]
                    )
                copy(vstack[:, 2 * j * P : (2 * j + 2) * P], ps[:, 0 : 2 * P])

            ot = [outp.tile([P, IC], bf16, name=f"ot{k}") for k in range(UT)]

            def mm(t, n):
                po = mmp.tile([P, NBLK], f32)
                nc.tensor.matmul(
                    po[:],
                    lhsT=ustack[:, t * P : (t + 1) * P],
                    rhs=vstack[:, n * NBLK : (n + 1) * NBLK],
                    start=True,
                    stop=True,
                )
                copy(ot[t][:, n * NBLK : (n + 1) * NBLK], po[:])
                # flush [0:1024) after block 1, then each later block alone,
                # spreading output DMA instead of bunching it in the tail
                if n >= 1:
                    sl = slice(0 if n == 1 else n * NBLK, (n + 1) * NBLK)
                    nc.sync.dma_start(
                        out=out_dram[t * P : (t + 1) * P, sl],
                        in_=ot[t][:, sl],
                    )

            # --- gather schedule: item tiles 0-7 first (unlocks matmul
            # blocks 0-1 and the first output flushes early), then users
            # trickled between the remaining item tiles, ending on users so
            # the post-gather tail is small.
            SEQ = [
                ("i", 0), ("i", 1), ("i", 2), ("i", 3), ("u", 0),
                ("i", 4), ("i", 5), ("u", 1), ("i", 6), ("i", 7),
                ("u", 2), ("i", 8), ("i", 9), ("u", 3), ("i", 10),
                ("i", 11), ("u", 4), ("i", 12), ("i", 13), ("u", 5),
                ("i", 14), ("i", 15), ("u", 6), ("u", 7),
            ]
            for kind, j in SEQ:
                if kind == "u":
                    gather(gu[j], ut_dram, j)
                else:
                    gather(gv[j], it_dram, UT + j)

            done = set()

            def emit_ready(users_done, items_done):
                blocks = min(NB, items_done // 4)
                for t in range(users_done):
                    for n in range(blocks):
                        if (t, n) not in done:
                            done.add((t, n))
                            mm(t, n)

            users_done = items_done = 0
            for kind, j in SEQ:
                if kind == "u":
                    u_transpose(j)
                    users_done = j + 1
                else:
                    if j % 2 == 1:
                        i_transpose_pair(j // 2)
                    items_done = j + 1
                emit_ready(users_done, items_done)

    nc.finalize()
    return nc


def kernel(user_hiddens, item_hiddens, user_ids, item_ids, **_):
    user_hiddens = np.ascontiguousarray(user_hiddens, dtype=np.float32)
    item_hiddens = np.ascontiguousarray(item_hiddens, dtype=np.float32)
    user_ids = np.asarray(user_ids)
    item_ids = np.asarray(item_ids)

    if "nc" not in _cache:
        _cache["nc"] = _build()
    nc = _cache["nc"]

    in_maps = []
    for c in range(N_CORES):
        cu, ci = divmod(c, RI)
        uc = user_ids[cu * UC : (cu + 1) * UC]
        icd = item_ids[ci * IC : (ci + 1) * IC]
        ids_t = np.empty((P, UT + IT), dtype=np.int32)
        ids_t[:, :UT] = uc.astype(np.int32).reshape(UT, P).T
        ids_t[:, UT:] = icd.astype(np.int32).reshape(IT, P).T
        in_maps.append(
            {
                "user_table": user_hiddens,
                "item_table": item_hiddens,
                "ids": np.ascontiguousarray(ids_t),
            }
        )

    res = run_bass_kernel_spmd(nc, in_maps, list(range(N_CORES)))
    out = np.empty((BU, BI), dtype=np.float32)
    for c in range(N_CORES):
        cu, ci = divmod(c, RI)
        out[cu * UC : (cu + 1) * UC, ci * IC : (ci + 1) * IC] = np.asarray(
            res.results[c]["out"]
        ).astype(np.float32)
    return out
